# revision 23
# baseline (speedup 1.0000x reference)
"""Linear-chain CRF forward pass on 8 Trainium2 NeuronCores.

Reference recurrence (per batch element b):
    alpha_t[j] = x_t[j] + logsumexp_k(alpha_{t-1}[k] + trans[j,k])
    out[b] = sum_j alpha_{L_b - 1}[j]

Device formulation: exp space with a constant per-step log shift c folded
into the transition matrix:
    E_t = (Mc @ E_{t-1}) * X_t,  Mc[j,k] = exp(trans[j,k] - c),  X_t = exp(x_t)
so alpha_t = log E_r + r*c + A for a per-trajectory constant A.

The 2048-step serial chain is broken via the Birkhoff contraction of the
positive map E -> Mc @ E (contraction <= tanh(spread(trans)/2) ~ 0.46 per
step; elementwise positive scalings are Hilbert-metric isometries): time is
cut into 32 segments of 64 steps, each warmed up W rounds from an arbitrary
positive init.  The unknown per-segment offsets A_s are recovered on the
host by telescoping mean log-ratios at segment boundaries, where the value
is computed by both the owning segment and its predecessor.

Per-core layout (32 batch elements/core, data-parallel over batch):
  State E[row, col]: 128 partitions = 2 row-blocks x 64 classes, 256
  columns = 8 segment-blocks x 32 local b.  Two independent instruction
  chains ("pairs"), each advanced per round by one K=128 block-diagonal
  float32r matmul (N=256) plus one (128,256) DVE multiply.
  Segment s = 16*pair + 8*rowblock + block.
  Segment 0 replays the exact trajectory from t=0 (true init
  exp(x_0 + origination) injected via its round-0 X columns).
"""

from contextlib import ExitStack

import numpy as np

B, T, C = 256, 2048, 64
NCORES = 8
BPC = B // NCORES          # 32
SEG = 32
SEG_LEN = T // SEG         # 64
W = 16                     # warmup rounds for segments s >= 1
L = SEG_LEN + W + 1        # 81 rounds; round 0 = init
PAIRS = 2
NCOL = 256
CHUNK = 9                  # rounds per DMA+exp chunk; L == 9 * CHUNK
SNAP_ROUNDS = (W, SEG_LEN, SEG_LEN + W)

_CACHE = {}


def _c_step(transitions, pad_x):
    """Mean per-step growth of max_j alpha, from a short host simulation."""
    x = np.asarray(pad_x[:4], np.float64)
    tr = np.asarray(transitions, np.float64)
    a = x[:, 0, :]
    tot, n = 0.0, 0
    for t in range(1, 257):
        s = a[:, None, :] + tr[None, :, :]
        m = s.max(axis=2, keepdims=True)
        a_new = x[:, t, :] + np.log(np.exp(s - m).sum(axis=2)) + m[:, :, 0]
        tot += float((a_new.max(axis=1) - a.max(axis=1)).mean())
        n += 1
        a = a_new
    return tot / n


def _seg_of(t_star):
    return min(t_star // SEG_LEN, SEG - 1)


def _round_of(t_star):
    s = _seg_of(t_star)
    return t_star if s == 0 else t_star - s * SEG_LEN + W


def _col_of(s, b=0):
    p, rem = divmod(s, 16)
    h, q = divmod(rem, 8)
    return p, h, q * 32 + b


def _build_host_inputs(pad_x, transitions, origination, c):
    """X_raw per core: [PAIRS, 128, L*NCOL] f32 laid out so each partition
    row is contiguous over (round, col); exp is applied on device.  Also the
    block-diagonal lhsT weights [128, 128] f32."""
    mc = np.exp(np.asarray(transitions, np.float64) - c).astype(np.float32)
    wmat = np.zeros((128, 128), np.float32)
    wmat[:64, :64] = mc.T      # lhsT[k, j] = Mc[j, k]
    wmat[64:, 64:] = mc.T

    x0 = np.asarray(pad_x, np.float32).copy()
    x0[:, 0, :] += np.asarray(origination, np.float32)[None, :]
    xc = x0.reshape(NCORES, BPC, T, C)

    xraw = np.zeros((NCORES, PAIRS, 128, L, NCOL), np.float32)
    for s in range(SEG):
        t0 = 0 if s == 0 else s * SEG_LEN - W
        t_idx = np.arange(L) + t0
        valid = (t_idx >= 0) & (t_idx < T)
        t_clip = np.clip(t_idx, 0, T - 1)
        p, h, col0 = _col_of(s)
        # (core, b, L, C) -> (core, C, L, b)
        blk = xc[:, :, t_clip, :] * valid[None, None, :, None]
        xraw[:, p, 64 * h:64 * h + 64, :, col0:col0 + 32] = \
            blk.transpose(0, 3, 2, 1)
    return xraw.reshape(NCORES, PAIRS, 128, L * NCOL), wmat


def _extraction_schedule(batch_sizes):
    """Per-core static extraction events (round, pair, rowblock, col,
    global_b).  The SPMD program is shared, so the device executes the
    union of all cores' events (each into its own fin column, keyed by
    global b); each core's host-side readback uses only its own events."""
    bs = np.asarray(batch_sizes).reshape(NCORES, BPC)
    sched = []
    for core in range(NCORES):
        ev = []
        for b in range(BPC):
            t_star = int(bs[core, b]) - 1
            s = _seg_of(t_star)
            r = _round_of(t_star)
            p, h, col = _col_of(s, b)
            ev.append((r, p, h, col, core * BPC + b))
        sched.append(ev)
    return sched


def _build_program(by_round):
    """Raw-bass program with explicit per-engine streams and standalone
    semaphore waits (DVE instructions only support ONE embedded sync wait on
    this toolchain, so Tile's embedded-wait scheduling cannot compile the
    tight mm->mul loop).  by_round: round -> [(p, h, col, global_b)].

    Engine streams:
      SP   : weight DMA, X chunk DMAs, snapshot DMAs, final fin DMA
      ACT  : f32r rounding copy of weights, exp of X chunks
      PE   : 2 block-diagonal f32r matmuls per round
      DVE  : 2 (128, NCOL) multiplies per round + fin column copies
    """
    import concourse.bass as bass
    from concourse import mybir

    dt = mybir.dt
    NCH = L // CHUNK          # 9 chunks
    ERING = 8
    nc = bass.Bass()
    xp = nc.declare_dram_parameter("xp", [PAIRS, 128, L * NCOL], dt.float32,
                                   False)
    wm = nc.declare_dram_parameter("wm", [128, 128], dt.float32, False)
    snaps = nc.declare_dram_parameter(
        "snaps", [len(SNAP_ROUNDS) * PAIRS, 128, NCOL], dt.float32r, True)
    fin = nc.declare_dram_parameter("fin", [64, B], dt.float32r, True)

    with ExitStack() as ctx:
        def sb(name, shape, d):
            return ctx.enter_context(nc.sbuf_tensor(name, shape, d))
        wraw = sb("wraw", [128, 128], dt.float32)
        wr = sb("wr", [128, 128], dt.float32r)
        raw = [[sb(f"raw{i}_{p}", [128, CHUNK * NCOL], dt.float32)
                for p in range(PAIRS)] for i in range(2)]
        xr = [[sb(f"xr{i}_{p}", [128, CHUNK * NCOL], dt.float32r)
               for p in range(PAIRS)] for i in range(3)]
        et = [[sb(f"et{p}_{i}", [128, NCOL], dt.float32r)
               for i in range(ERING)] for p in range(PAIRS)]
        # write-once staging for segment-boundary snapshots; DVE fills them
        # right after the snapshot round, SP drains them at the end
        snapst = [sb(f"snapst{i}", [128, NCOL], dt.float32r)
                  for i in range(len(SNAP_ROUNDS) * PAIRS)]
        fin_t = sb("fin_t", [64, B], dt.float32r)
        ps = [[ctx.enter_context(
            nc.psum_tensor(f"ps{p}_{i}", [128, NCOL], dt.float32))
            for i in range(2)] for p in range(PAIRS)]
        s_w = ctx.enter_context(nc.semaphore("s_w"))
        s_x0 = ctx.enter_context(nc.semaphore("s_x0"))
        s_x1 = ctx.enter_context(nc.semaphore("s_x1"))
        s_x = (s_x0, s_x1)
        s_a = ctx.enter_context(nc.semaphore("s_a"))
        s_pe = ctx.enter_context(nc.semaphore("s_pe"))
        s_v = ctx.enter_context(nc.semaphore("s_v"))
        s_f = ctx.enter_context(nc.semaphore("s_f"))
        s_s = ctx.enter_context(nc.semaphore("s_s"))
        s_o = ctx.enter_context(nc.semaphore("s_o"))
        block = ctx.enter_context(nc.Block())

        def xsl(p, r):
            k, rr = divmod(r, CHUNK)
            return xr[k % 3][p][:, rr * NCOL:(rr + 1) * NCOL]

        def ecur(p, r):
            return et[p][r % ERING]

        def eprev(p, r):
            if r == 1:
                return xsl(p, 0)
            return ecur(p, r - 1)[:]

        nfin = sum(len(v) for v in by_round.values())

        @block.sync
        def _(sync):
            sync.dma_start(wraw[:], wm[:, :]).then_inc(s_w, 16)
            for k in range(NCH):
                for p in range(PAIRS):
                    n = 2 * k + p
                    if k >= 2:
                        # raw slot reused; exp(k-2, p) must be done
                        sync.wait_ge(s_a, 2 * (k - 2) + p + 2)
                    if n >= 2:
                        # DMA-completion ordering protocol for the shared sem
                        sync.wait_ge(s_x[n % 2], 16 * (n // 2))
                    sync.dma_start(
                        raw[k % 2][p][:],
                        xp[p, :, k * CHUNK * NCOL:(k + 1) * CHUNK * NCOL],
                    ).then_inc(s_x[n % 2], 16)
            for i in range(len(snapst)):
                sync.wait_ge(s_s, i + 1)
                if i:
                    sync.wait_ge(s_o, 16 * i)
                sync.dma_start(snaps[i], snapst[i][:]).then_inc(s_o, 16)
            sync.wait_ge(s_f, nfin)
            sync.wait_ge(s_o, 16 * len(snapst))
            sync.dma_start(fin[:, :], fin_t[:]).then_inc(s_o, 16)

        @block.scalar
        def _(scalar):
            scalar.wait_ge(s_w, 16)
            nc.scalar.copy(wr[:], wraw[:]).then_inc(s_a, 1)
            for k in range(NCH):
                for p in range(PAIRS):
                    n = 2 * k + p
                    scalar.wait_ge(s_x[n % 2], 16 * (n // 2 + 1))
                    if k >= 3:
                        # xr slot reused; all muls of chunk k-3 done
                        scalar.wait_ge(s_v, 2 * (CHUNK * (k - 3) + CHUNK - 1))
                    nc.scalar.activation(
                        xr[k % 3][p][:], raw[k % 2][p][:],
                        mybir.ActivationFunctionType.Exp).then_inc(s_a, 1)

        @block.tensor
        def _(tensor):
            for r in range(1, L):
                for p in range(PAIRS):
                    if r == 1:
                        tensor.wait_ge(s_a, p + 2)
                    else:
                        tensor.wait_ge(s_v, 2 * (r - 2) + p + 1)
                    nc.tensor.matmul(ps[p][r % 2][:], wr[:], eprev(p, r),
                                     start=True, stop=True).then_inc(s_pe, 1)

        @block.vector
        def _(vector):
            for (p, h, col, gb) in by_round.get(0, ()):
                vector.wait_ge(s_a, p + 2)
                nc.vector.tensor_copy(
                    fin_t[:, gb:gb + 1],
                    xr[0][p][64 * h:64 * h + 64, col:col + 1],
                ).then_inc(s_f, 1)
            for r in range(1, L):
                for p in range(PAIRS):
                    vector.wait_ge(s_pe, 2 * (r - 1) + p + 1)
                    if r == 1 or r % CHUNK == 0:
                        vector.wait_ge(s_a, 2 * (r // CHUNK) + p + 2)
                    nc.vector.tensor_mul(ecur(p, r)[:],
                                         ps[p][r % 2][:],
                                         xsl(p, r)).then_inc(s_v, 1)
                if by_round.get(r) or r in SNAP_ROUNDS:
                    # DVE is pipelined: reads of a tile written by an
                    # earlier DVE instruction need the writer retired.
                    vector.wait_ge(s_v, 2 * r)
                if r in SNAP_ROUNDS:
                    si = SNAP_ROUNDS.index(r)
                    for p in range(PAIRS):
                        nc.vector.tensor_copy(
                            snapst[2 * si + p][:],
                            ecur(p, r)[:]).then_inc(s_s, 1)
                for (p, h, col, gb) in by_round.get(r, ()):
                    nc.vector.tensor_copy(
                        fin_t[:, gb:gb + 1],
                        ecur(p, r)[64 * h:64 * h + 64, col:col + 1],
                    ).then_inc(s_f, 1)

    return nc


def _postprocess(snaps, fin, sched_core, c):
    """Per-core host math (float64): stitch segment offsets, read finals."""
    ls = np.log(np.maximum(np.asarray(snaps, np.float64), 1e-300))
    snap = {W: ls[0:2], SEG_LEN: ls[2:4], SEG_LEN + W: ls[4:6]}

    def seg_cols(arr, s):
        p, h, col0 = _col_of(s)
        return arr[p][64 * h:64 * h + 64, col0:col0 + 32]  # (64, 32)

    A = np.zeros((SEG, BPC))
    for s in range(1, SEG):
        if s == 1:
            prev, i_prev = seg_cols(snap[SEG_LEN], 0), SEG_LEN
        else:
            prev, i_prev = seg_cols(snap[SEG_LEN + W], s - 1), SEG_LEN + W
        cur = seg_cols(snap[W], s)
        d = (prev + i_prev * c) - (cur + W * c)
        A[s] = A[s - 1] + d.mean(axis=0)

    lf = np.log(np.maximum(np.asarray(fin, np.float64), 1e-300))  # (64, B)
    res = np.empty(BPC)
    for (r, p, h, col, gb) in sched_core:
        s = 16 * p + 8 * h + col // 32
        b = gb % BPC
        res[b] = lf[:, gb].sum() + 64.0 * (r * c + A[s, b])
    return res


def kernel(pad_x, transitions, origination, batch_sizes):
    from concourse.bass_utils import run_bass_kernel_spmd

    pad_x = np.asarray(pad_x)
    transitions = np.asarray(transitions)
    origination = np.asarray(origination)
    batch_sizes = np.asarray(batch_sizes)

    c = _c_step(transitions, pad_x)
    xraw, wmat = _build_host_inputs(pad_x, transitions, origination, c)
    sched = _extraction_schedule(batch_sizes)

    by_round = {}
    for ev in sched:
        for (r, p, h, col, gb) in ev:
            by_round.setdefault(r, []).append((p, h, col, gb))

    key = (batch_sizes.tobytes(), round(float(c), 9))
    if key not in _CACHE:
        _CACHE[key] = _build_program(by_round)
    nc = _CACHE[key]

    in_maps = [{"xp": xraw[i], "wm": wmat} for i in range(NCORES)]
    out = run_bass_kernel_spmd(nc, in_maps, list(range(NCORES)))

    res = np.empty(B, np.float32)
    for i in range(NCORES):
        r = _postprocess(out.results[i]["snaps"], out.results[i]["fin"],
                         sched[i], c)
        res[i * BPC:(i + 1) * BPC] = r.astype(np.float32)
    return res
